# revision 31
# baseline (speedup 1.0000x reference)
# Trainium2 Bass kernel for nn_AdaptiveCrossHadamard.
#
# Reference computation (per sample):
#   y   = BN(Conv1x1(x))                                  [256, 64*64]
#   p   = mean_pixels(y); logits = conv1d(p, eca_w, k=5)  [256]
#   idx = top_32(logits) (sorted desc, ties -> lower idx)
#   xs  = y[idx]                                          [32, 4096]
#   z   = BN_s(xs[hi] * xs[hj])  for all i<j pairs        [496, 4096]
#   out = concat([y, z], channel axis)                    [752, 4096]
#
# Strategy (8 NeuronCores, batch-parallel, 2 samples/core, no collectives):
#   - BN folded into matmul weights host-side; ECA conv1d as a banded 256x256
#     matrix host-side (values from eca_w, structure static).
#   - y via fp16 matmuls (fp32 accumulate in PSUM), bias added by ScalarE on
#     the PSUM->SBUF copy.
#   - pooled computed EXACTLY in f32 via linearity: mean_pix(y) =
#     W' @ sum_pix(x)/4096 + b'  (f32 row-sums of x fused into the cast pass,
#     f32 matmul). This keeps the top-k ranking faithful to the f32 reference.
#   - top-k as dense ops: rank[c] = #{b!=c: logits[b] > logits[c]} (+ exact
#     tie-break via a lower-triangular mask), one-hot selection matrix S from
#     rank==iota, selected channels as a matmul with x (weights = W' @ S).
#   - pairwise Hadamard: one-hot pair matrices replicate the 32 selected rows
#     to 496 pair rows on the TensorEngine; VectorE does
#     t = (A * scale_s) * B in one scalar_tensor_tensor op (A read from PSUM);
#     the per-pair shift is added on DVE/ACT (alternating) into the staging
#     tile that feeds the output DMA.
#   - constants packed into 3 blob DMAs on the scalar HWDGE queue so the
#     x loads own the sync queue from t=0.
import os
import sys
import numpy as np

_TRN_REPO = "/opt/trn_rl_repo"
if _TRN_REPO not in sys.path and os.path.isdir(_TRN_REPO):
    sys.path.insert(0, _TRN_REPO)

import concourse.bacc as bacc
import concourse.bass as bass
import concourse.mybir as mybir
import concourse.tile as tile
from concourse.bass_utils import run_bass_kernel_spmd

F32 = mybir.dt.float32
F16 = mybir.dt.float16
AF = mybir.ActivationFunctionType
ALU = mybir.AluOpType

B, C1, H, W = 16, 256, 64, 64
PIX = H * W                      # 4096
CS = 32
CSE = CS * (CS - 1) // 2         # 496
NCORES = 8
SPC = B // NCORES                # samples per core = 2
COUT = C1 + CSE                  # 752
EPS = 1e-5

NT = PIX // 512                  # 8 pixel tiles of 512
MT4 = (CSE + 127) // 128         # 4 pair-row tiles (128,128,128,112)
HPIX = PIX // 2

# f32 const blob column layout: [128, CB32]
_WY32 = 0                        # wyT32s, 2 chunks x 256
_CMAT = 512                      # cmat, 2 x 256
_TRIL = 1024                     # tril, 2 x 256
_OFFD = 1536                     # offd, 2 x 256
_BCOL = 2048                     # bcol, 2 x 1
_SCOL = 2050                     # scale_s, 4 x 1
_SHCOL = 2054                    # shift_s, 4 x 1
_IOTA = 2058                     # iota, 32
CB32 = 2090
# fp16 const blob: [128, CB16]
_WY16 = 0                        # wyT16, 2 x 256
_WFOLD = 512                     # wfold16, 2 x 256
_BC16 = 1024                     # bcol16, 2 x 1
CB16 = 1026


def _build(nc: bass.Bass, dbg: bool = False):
    """Emit the per-core Tile program. SPMD: all 8 cores run this graph."""
    x_d = nc.dram_tensor("x", [SPC * C1, PIX], F32, kind="ExternalInput")
    out_d = nc.dram_tensor("out", [SPC * COUT, PIX], F32, kind="ExternalOutput")
    cb32_d = nc.dram_tensor("cb32", [128, CB32], F32, kind="ExternalInput")
    cb16_d = nc.dram_tensor("cb16", [128, CB16], F16, kind="ExternalInput")
    cbp_d = nc.dram_tensor("cbp", [CS + 1, 2 * CSE], F16, kind="ExternalInput")
    if dbg:
        dbg_pooled = nc.dram_tensor("dbg_pooled", [SPC * C1, 1], F32,
                                    kind="ExternalOutput")
        dbg_lrow = nc.dram_tensor("dbg_lrow", [SPC, C1], F32,
                                  kind="ExternalOutput")
        dbg_rank = nc.dram_tensor("dbg_rank", [SPC * C1, 1], F32,
                                  kind="ExternalOutput")
        dbg_st = nc.dram_tensor("dbg_st", [SPC * C1, CS], F16,
                                kind="ExternalOutput")
        dbg_xsel = nc.dram_tensor("dbg_xsel", [SPC * CS, PIX], F16,
                                  kind="ExternalOutput")
        dbg_t = nc.dram_tensor("dbg_t", [SPC * 512, PIX], F32,
                               kind="ExternalOutput")

    from contextlib import ExitStack
    with tile.TileContext(nc) as tc, ExitStack() as ctx:
        cpool = ctx.enter_context(tc.tile_pool(name="consts", bufs=1))
        x32p = ctx.enter_context(tc.tile_pool(name="x32", bufs=2))
        x16p = ctx.enter_context(tc.tile_pool(name="x16", bufs=4))
        yp = ctx.enter_context(tc.tile_pool(name="ysb", bufs=2))
        zp = ctx.enter_context(tc.tile_pool(name="zout", bufs=3))
        sqp = ctx.enter_context(tc.tile_pool(name="sq16", bufs=4))
        xselp = ctx.enter_context(tc.tile_pool(name="xsel", bufs=2))
        gp = ctx.enter_context(tc.tile_pool(name="gwork", bufs=2))
        smallp = ctx.enter_context(tc.tile_pool(name="small", bufs=4))
        # PSUM: pair-phase A/B in [128,1024] tiles (2 banks each, bufs=3)
        # partitioned from the y/xsel/small matmuls so neither starves the
        # other.
        psMM = ctx.enter_context(tc.tile_pool(name="psMM", bufs=2, space="PSUM"))
        psS = ctx.enter_context(tc.tile_pool(name="psS", bufs=3, space="PSUM"))
        psQ = ctx.enter_context(tc.tile_pool(name="psQ", bufs=3, space="PSUM"))

        dma = nc.sync.dma_start

        # ---- PE warmup: trip the HAM clock gate with ~5us of dense dummy
        # matmuls on a zero tile so real matmuls run at 2.4 GHz, not 1.2.
        warm = cpool.tile([128, 512], F16, tag="warm")
        nc.vector.memset(warm[:], 0.0)
        wps = psMM.tile([128, 512], F32, tag="mm", name="warm_ps")
        for _ in range(10):
            nc.tensor.matmul(wps[:], lhsT=warm[:, 0:128], rhs=warm[:],
                             start=True, stop=True)

        # ---- x loads FIRST (own the sync DMA queue), halves for early cast
        X32 = [[None] * 2 for _ in range(SPC)]
        for s in range(SPC):
            for kt in range(2):
                x32 = x32p.tile([128, PIX], F32)
                r0 = s * C1 + kt * 128
                dma(out=x32[:, :HPIX], in_=x_d[r0:r0 + 128, :HPIX])
                dma(out=x32[:, HPIX:], in_=x_d[r0:r0 + 128, HPIX:])
                X32[s][kt] = x32

        # ---- constants: 3 blob DMAs on the scalar HWDGE queue ----
        cb32 = cpool.tile([128, CB32], F32, tag="cb32")
        nc.scalar.dma_start(out=cb32[:], in_=cb32_d[:, :])
        cb16 = cpool.tile([128, CB16], F16, tag="cb16")
        nc.scalar.dma_start(out=cb16[:], in_=cb16_d[:, :])
        cbp = cpool.tile([CS + 1, 2 * CSE], F16, tag="cbp")
        nc.scalar.dma_start(out=cbp[:], in_=cbp_d[:, :])

        def c32(col, w):
            return cb32[:, col:col + w]

        wyT32s = [c32(_WY32 + k * 256, 256) for k in range(2)]
        cmat = [c32(_CMAT + k * 256, 256) for k in range(2)]
        tril = [c32(_TRIL + k * 256, 256) for k in range(2)]
        offd = [c32(_OFFD + k * 256, 256) for k in range(2)]
        bcol = [c32(_BCOL + k, 1) for k in range(2)]
        scol = [c32(_SCOL + m, 1) for m in range(MT4)]
        shcol = [c32(_SHCOL + m, 1) for m in range(MT4)]
        iota32 = c32(_IOTA, CS)
        wyT16 = [cb16[:, _WY16 + k * 256: _WY16 + (k + 1) * 256] for k in range(2)]
        wfold16 = [cb16[:, _WFOLD + k * 256: _WFOLD + (k + 1) * 256] for k in range(2)]
        bcol16 = [cb16[:, _BC16 + k: _BC16 + k + 1] for k in range(2)]
        pS16 = cbp[0:CS, 0:CSE]
        pQ16 = cbp[0:CS + 1, CSE:2 * CSE]

        X16 = [[None] * 2 for _ in range(SPC)]
        XSUM = [[None] * 2 for _ in range(SPC)]
        WSEL = [None] * SPC
        SBIAS = [None] * SPC
        XSEL = [None] * SPC
        XSQ = [None] * SPC

        def ph_cast(s):
            # fp16 cast + exact f32 row-sums (DVE, fused), by pixel half
            for kt in range(2):
                x32 = X32[s][kt]
                xs = smallp.tile([128, 2], F32, tag="xsum")
                xt = x16p.tile([128, PIX], F16)
                for h in range(2):
                    nc.vector.tensor_scalar(
                        xt[:, h * HPIX:(h + 1) * HPIX],
                        x32[:, h * HPIX:(h + 1) * HPIX], 1.0, 0.0,
                        op0=ALU.mult, op1=ALU.add,
                        accum_out=xs[:, h:h + 1])
                X16[s][kt] = xt
                XSUM[s][kt] = xs

        def ph_y(s):
            # y = W'x + b' (fp16 matmul, f32 psum), ACT adds bias.
            # nt pairs share one weight load per K-tile.
            for mt in range(2):
                y_sb = yp.tile([128, PIX], F32)
                for ntp in range(NT // 2):
                    yps = [psMM.tile([128, 512], F32, tag="mm", name=f"yps{j}")
                           for j in range(2)]
                    for kt in range(2):
                        for j in range(2):
                            nt = ntp * 2 + j
                            nc.tensor.matmul(
                                yps[j][:],
                                lhsT=wyT16[kt][:, mt * 128:(mt + 1) * 128],
                                rhs=X16[s][kt][:, nt * 512:(nt + 1) * 512],
                                start=(kt == 0), stop=(kt == 1))
                    for j in range(2):
                        nt = ntp * 2 + j
                        nc.scalar.activation(
                            y_sb[:, nt * 512:(nt + 1) * 512], yps[j][:],
                            AF.Identity, bias=bcol[mt], scale=1.0)
                dma(out=out_d[s * COUT + mt * 128: s * COUT + (mt + 1) * 128, :],
                    in_=y_sb[:])

        def ph_sel(s):
            # pooled = W'@xbar + b' (exact f32; wyT32s folds the /4096)
            pooled = []
            for mt in range(2):
                pp = psMM.tile([128, 2], F32, tag="mm")
                for kt in range(2):
                    nc.tensor.matmul(
                        pp[:], lhsT=wyT32s[kt][:, mt * 128:(mt + 1) * 128],
                        rhs=XSUM[s][kt][:], start=(kt == 0), stop=(kt == 1))
                ph = smallp.tile([128, 1], F32, tag="ph")
                nc.vector.tensor_reduce(ph[:], pp[:],
                                        axis=mybir.AxisListType.X, op=ALU.add)
                pb = smallp.tile([128, 1], F32, tag="pooled")
                nc.scalar.activation(pb[:], ph[:], AF.Identity,
                                     bias=bcol[mt], scale=1.0)
                pooled.append(pb)
                if dbg:
                    dma(out=dbg_pooled[s * C1 + mt * 128:
                                       s * C1 + (mt + 1) * 128, :], in_=pb[:])

            lr_ps = psMM.tile([1, C1], F32, tag="mm")
            for ot in range(2):
                nc.tensor.matmul(lr_ps[:], lhsT=pooled[ot][:], rhs=cmat[ot],
                                 start=(ot == 0), stop=(ot == 1))
            lrow = smallp.tile([1, C1], F32, tag="lrow")
            nc.scalar.copy(lrow[:], lr_ps[:])
            if dbg:
                dma(out=dbg_lrow[s:s + 1, :], in_=lrow[:])

            st = []
            for qt in range(2):
                lc_ps = psMM.tile([128, 1], F32, tag="mm")
                for ot in range(2):
                    nc.tensor.matmul(
                        lc_ps[:], lhsT=cmat[ot][:, qt * 128:(qt + 1) * 128],
                        rhs=pooled[ot][:], start=(ot == 0), stop=(ot == 1))
                lcol = smallp.tile([128, 1], F32, tag="lcol")
                nc.scalar.copy(lcol[:], lc_ps[:])

                # exact broadcast of logits row to all partitions (no PE fp32
                # rounding: the fp32 PE path is ~1e-7 lossy, which made the
                # diagonal compare Brow[a,a] vs lcol[a] misfire)
                brow = gp.tile([128, C1], F32, tag="brow")
                nc.gpsimd.partition_broadcast(brow[:], lrow[:])
                # rank[a] = #{b!=a: logits[b] > logits[a]}
                #        + #{b < a: logits[b] == logits[a]}   (jax tie-break)
                g2 = gp.tile([128, C1], F32)
                nc.vector.scalar_tensor_tensor(
                    g2[:], brow[:], lcol[:], tril[qt],
                    op0=ALU.is_equal, op1=ALU.mult)
                gsum = gp.tile([128, C1], F32)
                nc.vector.scalar_tensor_tensor(
                    gsum[:], brow[:], lcol[:], g2[:],
                    op0=ALU.is_gt, op1=ALU.add)
                gm = gp.tile([128, C1], F32)
                nc.vector.tensor_tensor(gm[:], gsum[:], offd[qt], op=ALU.mult)
                rank = smallp.tile([128, 1], F32, tag="rank")
                nc.vector.tensor_reduce(rank[:], gm[:],
                                        axis=mybir.AxisListType.X, op=ALU.add)
                # S_T[c, k] = (rank[c] == k)
                stq = smallp.tile([128, CS], F16, tag="st")
                nc.vector.tensor_scalar(stq[:], iota32, rank[:], None,
                                        op0=ALU.is_equal)
                st.append(stq)
                if dbg:
                    r0 = s * C1 + qt * 128
                    dma(out=dbg_rank[r0:r0 + 128, :], in_=rank[:])
                    dma(out=dbg_st[r0:r0 + 128, :], in_=stq[:])

            # selection weights: W_selT[c,k] = sum_o W'[o,c] S_T[o,k]
            wsel = []
            for ct in range(2):
                ws_ps = psMM.tile([128, CS], F32, tag="mm")
                for ot in range(2):
                    nc.tensor.matmul(
                        ws_ps[:], lhsT=wfold16[ot][:, ct * 128:(ct + 1) * 128],
                        rhs=st[ot][:], start=(ot == 0), stop=(ot == 1))
                wsq = smallp.tile([128, CS], F16, tag="wsel")
                nc.scalar.copy(wsq[:], ws_ps[:])
                wsel.append(wsq)
            WSEL[s] = wsel
            sb_ps = psMM.tile([CS, 1], F32, tag="mm")
            for ot in range(2):
                nc.tensor.matmul(sb_ps[:], lhsT=st[ot][:], rhs=bcol16[ot],
                                 start=(ot == 0), stop=(ot == 1))
            sbias = smallp.tile([CS, 1], F32, tag="sbias")
            nc.scalar.copy(sbias[:], sb_ps[:])
            SBIAS[s] = sbias

        def ph_xsel(s):
            # x_sel = W_sel @ x + S b'  (fp16, straight from x)
            xsel = xselp.tile([CS, PIX], F16)
            for ntp in range(NT // 2):
                xps = [psMM.tile([CS, 512], F32, tag="mm", name=f"xps{j}") for j in range(2)]
                for kt in range(2):
                    for j in range(2):
                        nt = ntp * 2 + j
                        nc.tensor.matmul(
                            xps[j][:], lhsT=WSEL[s][kt][:],
                            rhs=X16[s][kt][:, nt * 512:(nt + 1) * 512],
                            start=(kt == 0), stop=(kt == 1))
                for j in range(2):
                    nt = ntp * 2 + j
                    nc.scalar.activation(xsel[:, nt * 512:(nt + 1) * 512],
                                         xps[j][:], AF.Identity,
                                         bias=SBIAS[s][:], scale=1.0)
            XSEL[s] = xsel
            # xsq_aug: rows 0-31 = xsel^2 (fp16), row 32 = 1.0 (carries the
            # -shift row of pQ through the Q matmul)
            xsq = xselp.tile([CS + 1, PIX], F16, tag="xsq", name="xsq")
            nc.vector.memset(xsq[CS:CS + 1, :], 1.0)
            for h in range(2):
                nc.scalar.activation(xsq[0:CS, h * HPIX:(h + 1) * HPIX],
                                     xsel[:, h * HPIX:(h + 1) * HPIX],
                                     AF.Square)
            XSQ[s] = xsq
            if dbg:
                dma(out=dbg_xsel[s * CS:(s + 1) * CS, :], in_=xsel[:])

        def ph_z(s, m, half):
            # z = Square(sqrt(s/2)(xi+xj) @ pS) - [ (s/2)(xi^2+xj^2) - shift ]
            # PE: S and Q matmuls; ACT: Square (doubles as PSUM->SBUF move);
            # DVE: one subtract into the output staging tile.
            p = min(128, CSE - m * 128)
            xsel = XSEL[s]
            xsq = XSQ[s]
            zo = zp.tile([128, HPIX], F32)
            for ntp in range(2):
                sps = [psS.tile([128, 512], F32, tag="sps", name=f"sps{j}")
                       for j in range(2)]
                qps = [psQ.tile([128, 512], F32, tag="qps", name=f"qps{j}")
                       for j in range(2)]
                for j in range(2):
                    nt = half * 4 + ntp * 2 + j
                    nc.tensor.matmul(sps[j][:p, :],
                                     lhsT=pS16[:, m * 128: m * 128 + p],
                                     rhs=xsel[:, nt * 512:(nt + 1) * 512],
                                     start=True, stop=True)
                for j in range(2):
                    nt = half * 4 + ntp * 2 + j
                    nc.tensor.matmul(qps[j][:p, :],
                                     lhsT=pQ16[:, m * 128: m * 128 + p],
                                     rhs=xsq[:, nt * 512:(nt + 1) * 512],
                                     start=True, stop=True)
                for j in range(2):
                    nt2 = ntp * 2 + j
                    sq = sqp.tile([128, 512], F16, tag="sq", name="sq")
                    nc.scalar.activation(sq[:p, :], sps[j][:p, :], AF.Square)
                    # z = (Q * -1) + sq  — same operand pattern as the
                    # hardware-proven stt (PSUM f32 in0, fp16 SBUF in1)
                    nc.vector.scalar_tensor_tensor(
                        zo[:p, nt2 * 512:(nt2 + 1) * 512],
                        qps[j][:p, :], -1.0, sq[:p, :],
                        op0=ALU.mult, op1=ALU.add)
            if dbg:
                dma(out=dbg_t[s * 512 + m * 128: s * 512 + m * 128 + p,
                              half * HPIX:(half + 1) * HPIX],
                    in_=zo[:p, :])
            r0 = s * COUT + C1 + m * 128
            dma(out=out_d[r0:r0 + p, half * HPIX:(half + 1) * HPIX],
                in_=zo[:p, :])

        # program order == scheduling priority: casts and selection chains
        # early, z phases as soon as each sample's xsel is ready, y matmuls
        # fill PE gaps (their inputs are ready from the start).
        ph_cast(0)
        ph_cast(1)
        ph_sel(0)
        ph_xsel(0)
        ph_sel(1)
        ph_z(0, 0, 0)
        ph_z(0, 0, 1)
        ph_y(0)
        ph_xsel(1)
        ph_z(0, 1, 0)
        ph_z(0, 1, 1)
        ph_z(1, 0, 0)
        ph_z(1, 0, 1)
        ph_y(1)
        ph_z(1, 1, 0)
        ph_z(1, 1, 1)
        for m, half in [(2, 0), (2, 1), (3, 0), (3, 1)]:
            ph_z(0, m, half)
            ph_z(1, m, half)


_CACHE = {}


def _get_nc(dbg: bool = False):
    key = f"nc{int(dbg)}"
    if key not in _CACHE:
        nc = bacc.Bacc("TRN2", target_bir_lowering=False, debug=False,
                       num_devices=NCORES)
        _build(nc, dbg=dbg)
        nc.compile()
        _CACHE[key] = nc
    return _CACHE[key]


def _host_params(w_fc, b_fc, g_x, b_x, m_x, v_x, eca_w, g_s, b_s, m_s, v_s):
    sx = (g_x / np.sqrt(v_x + EPS)).astype(np.float32)            # [256]
    Wp = (sx[:, None] * w_fc).astype(np.float32)                  # [o, c]
    bp = (sx * b_fc + b_x - m_x * sx).astype(np.float32)          # [256]

    cmat = np.zeros((C1, C1), np.float32)                         # [o, q]
    for k in range(5):
        d = k - 2                                                 # o - q
        for q in range(C1):
            o = q + d
            if 0 <= o < C1:
                cmat[o, q] = eca_w[k]

    tril = (np.arange(C1)[None, :] < np.arange(C1)[:, None]).astype(np.float32)

    hi, hj = np.triu_indices(CS, k=1)
    ss = (g_s / np.sqrt(v_s + EPS)).astype(np.float32)
    sh = (b_s - m_s * ss).astype(np.float32)
    # squares-trick pair matrices:
    #   S' = pS.T @ xsel with pS[i,pq] = sqrt(s/2) * [i in (hi,hj)]
    #   Q  = pQ.T @ [xsel^2; 1] with pQ[i,pq] = (s/2)*[i in (hi,hj)],
    #        pQ[32,pq] = -shift  =>  z = S'^2 - Q
    ar = np.arange(CSE)
    pS = np.zeros((CS, CSE), np.float32)
    pS[hi, ar] = 1.0
    pS[hj, ar] += 1.0
    pS16 = (pS * np.sqrt(ss / 2.0)[None, :]).astype(np.float16)
    pQ = np.zeros((CS + 1, CSE), np.float32)
    pQ[hi, ar] = 1.0
    pQ[hj, ar] += 1.0
    pQ[:CS] *= (ss / 2.0)[None, :]
    pQ[CS] = -sh
    pQ16 = pQ.astype(np.float16)

    return {
        "wyT16": Wp.T.astype(np.float16).copy(),
        "wyT32s": (Wp.T / PIX).astype(np.float32).copy(),
        "wfold16": Wp.astype(np.float16).copy(),
        "bcol": bp.reshape(C1, 1).copy(),
        "bcol16": bp.astype(np.float16).reshape(C1, 1).copy(),
        "cmat": cmat,
        "tril": tril,
        "offd": (1.0 - np.eye(C1, dtype=np.float32)),
        "iota32": np.tile(np.arange(CS, dtype=np.float32), (128, 1)).copy(),
        "pS16": pS16,
        "pQ16": pQ16,
        "scol": ss.reshape(CSE, 1).copy(),
        "shcol": sh.reshape(CSE, 1).copy(),
    }


def _semantic_params(inputs):
    return _host_params(
        np.asarray(inputs["w_fc"], np.float32),
        np.asarray(inputs["b_fc"], np.float32),
        np.asarray(inputs["bn_x_gamma"], np.float32),
        np.asarray(inputs["bn_x_beta"], np.float32),
        np.asarray(inputs["bn_x_mean"], np.float32),
        np.asarray(inputs["bn_x_var"], np.float32),
        np.asarray(inputs["eca_w"], np.float32),
        np.asarray(inputs["bn_s_gamma"], np.float32),
        np.asarray(inputs["bn_s_beta"], np.float32),
        np.asarray(inputs["bn_s_mean"], np.float32),
        np.asarray(inputs["bn_s_var"], np.float32),
    )


def _pack_blobs(P):
    """Pack semantic params into the 3 const blobs matching _build's layout."""
    cb32 = np.zeros((128, CB32), np.float32)
    for k in range(2):
        cb32[:, _WY32 + k * 256: _WY32 + (k + 1) * 256] = \
            P["wyT32s"][k * 128:(k + 1) * 128]
        cb32[:, _CMAT + k * 256: _CMAT + (k + 1) * 256] = \
            P["cmat"][k * 128:(k + 1) * 128]
        cb32[:, _TRIL + k * 256: _TRIL + (k + 1) * 256] = \
            P["tril"][k * 128:(k + 1) * 128]
        cb32[:, _OFFD + k * 256: _OFFD + (k + 1) * 256] = \
            P["offd"][k * 128:(k + 1) * 128]
        cb32[:, _BCOL + k] = P["bcol"][k * 128:(k + 1) * 128, 0]
    for m in range(MT4):
        p = min(128, CSE - m * 128)
        cb32[:p, _SCOL + m] = P["scol"][m * 128: m * 128 + p, 0]
        cb32[:p, _SHCOL + m] = P["shcol"][m * 128: m * 128 + p, 0]
    cb32[:, _IOTA:_IOTA + CS] = P["iota32"]

    cb16 = np.zeros((128, CB16), np.float16)
    for k in range(2):
        cb16[:, _WY16 + k * 256: _WY16 + (k + 1) * 256] = \
            P["wyT16"][k * 128:(k + 1) * 128]
        cb16[:, _WFOLD + k * 256: _WFOLD + (k + 1) * 256] = \
            P["wfold16"][k * 128:(k + 1) * 128]
        cb16[:, _BC16 + k] = P["bcol16"][k * 128:(k + 1) * 128, 0]

    cbp = np.zeros((CS + 1, 2 * CSE), np.float16)
    cbp[0:CS, 0:CSE] = P["pS16"]
    cbp[0:CS + 1, CSE:2 * CSE] = P["pQ16"]
    return {"cb32": cb32, "cb16": np.ascontiguousarray(cb16),
            "cbp": np.ascontiguousarray(cbp)}


def _in_maps(inputs):
    x = np.ascontiguousarray(np.asarray(inputs["x"], np.float32))
    blobs = _pack_blobs(_semantic_params(inputs))
    maps = []
    for c in range(NCORES):
        shard = x[c * SPC:(c + 1) * SPC].reshape(SPC * C1, PIX)
        maps.append({"x": np.ascontiguousarray(shard), **blobs})
    return maps


def _ensure_ntff_hook():
    """The agent image lacks antenv.axon_hooks; synthesize it so
    run_bass_kernel_spmd(trace=True) can reach the NTFF profiler in
    libaxon_pjrt.so. Safe no-op if anything is missing."""
    try:
        import antenv.axon_hooks  # noqa: F401
        return
    except ImportError:
        pass
    try:
        import types
        import antenv
        from trn_agent_boot.trn_boot import _ntff_profile_via_ctypes
        hook = _ntff_profile_via_ctypes("/opt/axon/libaxon_pjrt.so")
        mod = types.ModuleType("antenv.axon_hooks")
        mod._hook = hook
        mod.get_axon_ntff_profile_hook = lambda: mod._hook
        mod.set_axon_ntff_profile_hook = lambda h: setattr(mod, "_hook", h)
        sys.modules["antenv.axon_hooks"] = mod
        antenv.axon_hooks = mod
    except Exception as e:  # pragma: no cover
        print(f"ntff hook shim failed: {e}", file=sys.stderr)


def run(inputs, trace=False, dbg=False):
    if trace:
        _ensure_ntff_hook()
    nc = _get_nc(dbg=dbg)
    maps = _in_maps(inputs)
    res = run_bass_kernel_spmd(nc, maps, core_ids=list(range(NCORES)),
                               trace=trace)
    outs = [np.asarray(res.results[c]["out"], np.float32)
            .reshape(SPC, COUT, H, W) for c in range(NCORES)]
    return np.concatenate(outs, axis=0), res


def kernel(**inputs) -> np.ndarray:
    out, _ = run(inputs, trace=False)
    return out


# revision 32
# speedup vs baseline: 1.1959x; 1.1959x over previous
# Trainium2 Bass kernel for nn_AdaptiveCrossHadamard.
#
# Reference computation (per sample):
#   y   = BN(Conv1x1(x))                                  [256, 64*64]
#   p   = mean_pixels(y); logits = conv1d(p, eca_w, k=5)  [256]
#   idx = top_32(logits) (sorted desc, ties -> lower idx)
#   xs  = y[idx]                                          [32, 4096]
#   z   = BN_s(xs[hi] * xs[hj])  for all i<j pairs        [496, 4096]
#   out = concat([y, z], channel axis)                    [752, 4096]
#
# Strategy (8 NeuronCores, batch-parallel, 2 samples/core, no collectives):
#   - BN folded into matmul weights host-side; ECA conv1d as a banded 256x256
#     matrix host-side (values from eca_w, structure static).
#   - y via fp16 matmuls (fp32 accumulate in PSUM), bias added by ScalarE on
#     the PSUM->SBUF copy.
#   - pooled computed EXACTLY in f32 via linearity: mean_pix(y) =
#     W' @ sum_pix(x)/4096 + b'  (f32 row-sums of x fused into the cast pass,
#     f32 matmul). This keeps the top-k ranking faithful to the f32 reference.
#   - top-k as dense ops: rank[c] = #{b!=c: logits[b] > logits[c]} (+ exact
#     tie-break via a lower-triangular mask), one-hot selection matrix S from
#     rank==iota, selected channels as a matmul with x (weights = W' @ S).
#   - pairwise Hadamard: one-hot pair matrices replicate the 32 selected rows
#     to 496 pair rows on the TensorEngine; VectorE does
#     t = (A * scale_s) * B in one scalar_tensor_tensor op (A read from PSUM);
#     the per-pair shift is added on DVE/ACT (alternating) into the staging
#     tile that feeds the output DMA.
#   - constants packed into 3 blob DMAs on the scalar HWDGE queue so the
#     x loads own the sync queue from t=0.
import os
import sys
import numpy as np

_TRN_REPO = "/opt/trn_rl_repo"
if _TRN_REPO not in sys.path and os.path.isdir(_TRN_REPO):
    sys.path.insert(0, _TRN_REPO)

import concourse.bacc as bacc
import concourse.bass as bass
import concourse.mybir as mybir
import concourse.tile as tile
from concourse.bass_utils import run_bass_kernel_spmd

F32 = mybir.dt.float32
F16 = mybir.dt.float16
AF = mybir.ActivationFunctionType
ALU = mybir.AluOpType

B, C1, H, W = 16, 256, 64, 64
PIX = H * W                      # 4096
CS = 32
CSE = CS * (CS - 1) // 2         # 496
NCORES = 8
SPC = B // NCORES                # samples per core = 2
COUT = C1 + CSE                  # 752
EPS = 1e-5

NT = PIX // 512                  # 8 pixel tiles of 512
MT4 = (CSE + 127) // 128         # 4 pair-row tiles (128,128,128,112)
HPIX = PIX // 2

# f32 const blob column layout: [128, CB32]
_WY32 = 0                        # wyT32s, 2 chunks x 256
_CMAT = 512                      # cmat, 2 x 256
_TRIL = 1024                     # tril, 2 x 256
_OFFD = 1536                     # offd, 2 x 256
_BCOL = 2048                     # bcol, 2 x 1
_SCOL = 2050                     # scale_s, 4 x 1
_SHCOL = 2054                    # shift_s, 4 x 1
_IOTA = 2058                     # iota, 32
CB32 = 2090
# fp16 const blob: [128, CB16]
_WY16 = 0                        # wyT16, 2 x 256
_WFOLD = 512                     # wfold16, 2 x 256
_BC16 = 1024                     # bcol16, 2 x 1
CB16 = 1026


def _build(nc: bass.Bass, dbg: bool = False):
    """Emit the per-core Tile program. SPMD: all 8 cores run this graph."""
    x_d = nc.dram_tensor("x", [SPC * C1, PIX], F32, kind="ExternalInput")
    out_d = nc.dram_tensor("out", [SPC * COUT, PIX], F32, kind="ExternalOutput")
    cb32_d = nc.dram_tensor("cb32", [128, CB32], F32, kind="ExternalInput")
    cb16_d = nc.dram_tensor("cb16", [128, CB16], F16, kind="ExternalInput")
    cbp_d = nc.dram_tensor("cbp", [CS + 1, 2 * CSE], F16, kind="ExternalInput")
    ones_d = nc.dram_tensor("ones_pix", [1, PIX], F16, kind="ExternalInput")
    if dbg:
        dbg_pooled = nc.dram_tensor("dbg_pooled", [SPC * C1, 1], F32,
                                    kind="ExternalOutput")
        dbg_lrow = nc.dram_tensor("dbg_lrow", [SPC, C1], F32,
                                  kind="ExternalOutput")
        dbg_rank = nc.dram_tensor("dbg_rank", [SPC * C1, 1], F32,
                                  kind="ExternalOutput")
        dbg_st = nc.dram_tensor("dbg_st", [SPC * C1, CS], F16,
                                kind="ExternalOutput")
        dbg_xsel = nc.dram_tensor("dbg_xsel", [SPC * CS, PIX], F16,
                                  kind="ExternalOutput")
        dbg_t = nc.dram_tensor("dbg_t", [SPC * 512, PIX], F32,
                               kind="ExternalOutput")

    from contextlib import ExitStack
    with tile.TileContext(nc) as tc, ExitStack() as ctx:
        cpool = ctx.enter_context(tc.tile_pool(name="consts", bufs=1))
        x32p = ctx.enter_context(tc.tile_pool(name="x32", bufs=2))
        x16p = ctx.enter_context(tc.tile_pool(name="x16", bufs=4))
        yp = ctx.enter_context(tc.tile_pool(name="ysb", bufs=2))
        zp = ctx.enter_context(tc.tile_pool(name="zout", bufs=3))
        sqp = ctx.enter_context(tc.tile_pool(name="sq16", bufs=4))
        xselp = ctx.enter_context(tc.tile_pool(name="xsel", bufs=2))
        gp = ctx.enter_context(tc.tile_pool(name="gwork", bufs=2))
        smallp = ctx.enter_context(tc.tile_pool(name="small", bufs=4))
        # PSUM: pair-phase A/B in [128,1024] tiles (2 banks each, bufs=3)
        # partitioned from the y/xsel/small matmuls so neither starves the
        # other.
        psMM = ctx.enter_context(tc.tile_pool(name="psMM", bufs=2, space="PSUM"))
        psS = ctx.enter_context(tc.tile_pool(name="psS", bufs=3, space="PSUM"))
        psQ = ctx.enter_context(tc.tile_pool(name="psQ", bufs=3, space="PSUM"))

        dma = nc.sync.dma_start

        # ---- PE warmup: trip the HAM clock gate with ~5us of dense dummy
        # matmuls on a zero tile so real matmuls run at 2.4 GHz, not 1.2.
        warm = cpool.tile([128, 512], F16, tag="warm")
        nc.vector.memset(warm[:], 0.0)
        wps = psMM.tile([128, 512], F32, tag="mm", name="warm_ps")
        for _ in range(26):
            nc.tensor.matmul(wps[:], lhsT=warm[:, 0:128], rhs=warm[:],
                             start=True, stop=True)

        # ---- x loads FIRST (own the sync DMA queue), halves for early cast
        X32 = [[None] * 2 for _ in range(SPC)]
        for s in range(SPC):
            for kt in range(2):
                x32 = x32p.tile([128, PIX], F32)
                r0 = s * C1 + kt * 128
                dma(out=x32[:, :HPIX], in_=x_d[r0:r0 + 128, :HPIX])
                dma(out=x32[:, HPIX:], in_=x_d[r0:r0 + 128, HPIX:])
                X32[s][kt] = x32

        # ---- constants: 3 blob DMAs on the scalar HWDGE queue ----
        cb32 = cpool.tile([128, CB32], F32, tag="cb32")
        nc.scalar.dma_start(out=cb32[:], in_=cb32_d[:, :])
        cb16 = cpool.tile([128, CB16], F16, tag="cb16")
        nc.scalar.dma_start(out=cb16[:], in_=cb16_d[:, :])
        cbp = cpool.tile([CS + 1, 2 * CSE], F16, tag="cbp")
        nc.scalar.dma_start(out=cbp[:], in_=cbp_d[:, :])

        def c32(col, w):
            return cb32[:, col:col + w]

        wyT32s = [c32(_WY32 + k * 256, 256) for k in range(2)]
        cmat = [c32(_CMAT + k * 256, 256) for k in range(2)]
        tril = [c32(_TRIL + k * 256, 256) for k in range(2)]
        offd = [c32(_OFFD + k * 256, 256) for k in range(2)]
        bcol = [c32(_BCOL + k, 1) for k in range(2)]
        scol = [c32(_SCOL + m, 1) for m in range(MT4)]
        shcol = [c32(_SHCOL + m, 1) for m in range(MT4)]
        iota32 = c32(_IOTA, CS)
        wyT16 = [cb16[:, _WY16 + k * 256: _WY16 + (k + 1) * 256] for k in range(2)]
        wfold16 = [cb16[:, _WFOLD + k * 256: _WFOLD + (k + 1) * 256] for k in range(2)]
        bcol16 = [cb16[:, _BC16 + k: _BC16 + k + 1] for k in range(2)]
        pS16 = cbp[0:CS, 0:CSE]
        pQ16 = cbp[0:CS + 1, CSE:2 * CSE]

        X16 = [[None] * 2 for _ in range(SPC)]
        XSUM = [[None] * 2 for _ in range(SPC)]
        WSEL = [None] * SPC
        SBIAS = [None] * SPC
        XSEL = [None] * SPC
        XSQ = [None] * SPC

        def ph_cast(s):
            # fp16 cast + exact f32 row-sums, by pixel half:
            # kt0 on DVE (fused tensor_scalar+accum), kt1 on ACT (Copy+accum)
            for kt in range(2):
                x32 = X32[s][kt]
                xs = smallp.tile([128, 2], F32, tag="xsum")
                xt = x16p.tile([128, PIX], F16)
                for h in range(2):
                    dst = xt[:, h * HPIX:(h + 1) * HPIX]
                    srcv = x32[:, h * HPIX:(h + 1) * HPIX]
                    acc = xs[:, h:h + 1]
                    if kt == 0:
                        nc.vector.tensor_scalar(dst, srcv, 1.0, 0.0,
                                                op0=ALU.mult, op1=ALU.add,
                                                accum_out=acc)
                    else:
                        nc.scalar.activation(dst, srcv, AF.Copy,
                                             accum_out=acc)
                X16[s][kt] = xt
                XSUM[s][kt] = xs

        def ph_y(s):
            # y = W'x + b' (fp16 matmul, f32 psum), ACT adds bias.
            # nt pairs share one weight load per K-tile.
            for mt in range(2):
                y_sb = yp.tile([128, PIX], F32)
                for ntp in range(NT // 2):
                    yps = [psMM.tile([128, 512], F32, tag="mm", name=f"yps{j}")
                           for j in range(2)]
                    for kt in range(2):
                        for j in range(2):
                            nt = ntp * 2 + j
                            nc.tensor.matmul(
                                yps[j][:],
                                lhsT=wyT16[kt][:, mt * 128:(mt + 1) * 128],
                                rhs=X16[s][kt][:, nt * 512:(nt + 1) * 512],
                                start=(kt == 0), stop=(kt == 1))
                    for j in range(2):
                        nt = ntp * 2 + j
                        nc.scalar.activation(
                            y_sb[:, nt * 512:(nt + 1) * 512], yps[j][:],
                            AF.Identity, bias=bcol[mt], scale=1.0)
                dma(out=out_d[s * COUT + mt * 128: s * COUT + (mt + 1) * 128, :],
                    in_=y_sb[:])

        def ph_sel(s):
            # pooled = W'@xbar + b' (exact f32; wyT32s folds the /4096)
            pooled = []
            for mt in range(2):
                pp = psMM.tile([128, 2], F32, tag="mm")
                for kt in range(2):
                    nc.tensor.matmul(
                        pp[:], lhsT=wyT32s[kt][:, mt * 128:(mt + 1) * 128],
                        rhs=XSUM[s][kt][:], start=(kt == 0), stop=(kt == 1))
                ph = smallp.tile([128, 1], F32, tag="ph")
                nc.vector.tensor_reduce(ph[:], pp[:],
                                        axis=mybir.AxisListType.X, op=ALU.add)
                pb = smallp.tile([128, 1], F32, tag="pooled")
                nc.scalar.activation(pb[:], ph[:], AF.Identity,
                                     bias=bcol[mt], scale=1.0)
                pooled.append(pb)
                if dbg:
                    dma(out=dbg_pooled[s * C1 + mt * 128:
                                       s * C1 + (mt + 1) * 128, :], in_=pb[:])

            lr_ps = psMM.tile([1, C1], F32, tag="mm")
            for ot in range(2):
                nc.tensor.matmul(lr_ps[:], lhsT=pooled[ot][:], rhs=cmat[ot],
                                 start=(ot == 0), stop=(ot == 1))
            lrow = smallp.tile([1, C1], F32, tag="lrow")
            nc.scalar.copy(lrow[:], lr_ps[:])
            if dbg:
                dma(out=dbg_lrow[s:s + 1, :], in_=lrow[:])

            st = []
            for qt in range(2):
                lc_ps = psMM.tile([128, 1], F32, tag="mm")
                for ot in range(2):
                    nc.tensor.matmul(
                        lc_ps[:], lhsT=cmat[ot][:, qt * 128:(qt + 1) * 128],
                        rhs=pooled[ot][:], start=(ot == 0), stop=(ot == 1))
                lcol = smallp.tile([128, 1], F32, tag="lcol")
                nc.scalar.copy(lcol[:], lc_ps[:])

                # exact broadcast of logits row to all partitions (no PE fp32
                # rounding: the fp32 PE path is ~1e-7 lossy, which made the
                # diagonal compare Brow[a,a] vs lcol[a] misfire)
                brow = gp.tile([128, C1], F32, tag="brow")
                nc.gpsimd.partition_broadcast(brow[:], lrow[:])
                # rank[a] = #{b!=a: logits[b] > logits[a]}
                #        + #{b < a: logits[b] == logits[a]}   (jax tie-break)
                g2 = gp.tile([128, C1], F32)
                nc.vector.scalar_tensor_tensor(
                    g2[:], brow[:], lcol[:], tril[qt],
                    op0=ALU.is_equal, op1=ALU.mult)
                gsum = gp.tile([128, C1], F32)
                nc.vector.scalar_tensor_tensor(
                    gsum[:], brow[:], lcol[:], g2[:],
                    op0=ALU.is_gt, op1=ALU.add)
                gm = gp.tile([128, C1], F32)
                nc.vector.tensor_tensor(gm[:], gsum[:], offd[qt], op=ALU.mult)
                rank = smallp.tile([128, 1], F32, tag="rank")
                nc.vector.tensor_reduce(rank[:], gm[:],
                                        axis=mybir.AxisListType.X, op=ALU.add)
                # S_T[c, k] = (rank[c] == k)
                stq = smallp.tile([128, CS], F16, tag="st")
                nc.vector.tensor_scalar(stq[:], iota32, rank[:], None,
                                        op0=ALU.is_equal)
                st.append(stq)
                if dbg:
                    r0 = s * C1 + qt * 128
                    dma(out=dbg_rank[r0:r0 + 128, :], in_=rank[:])
                    dma(out=dbg_st[r0:r0 + 128, :], in_=stq[:])

            # selection weights: W_selT[c,k] = sum_o W'[o,c] S_T[o,k]
            wsel = []
            for ct in range(2):
                ws_ps = psMM.tile([128, CS], F32, tag="mm")
                for ot in range(2):
                    nc.tensor.matmul(
                        ws_ps[:], lhsT=wfold16[ot][:, ct * 128:(ct + 1) * 128],
                        rhs=st[ot][:], start=(ot == 0), stop=(ot == 1))
                wsq = smallp.tile([128, CS], F16, tag="wsel")
                nc.scalar.copy(wsq[:], ws_ps[:])
                wsel.append(wsq)
            WSEL[s] = wsel
            sb_ps = psMM.tile([CS, 1], F32, tag="mm")
            for ot in range(2):
                nc.tensor.matmul(sb_ps[:], lhsT=st[ot][:], rhs=bcol16[ot],
                                 start=(ot == 0), stop=(ot == 1))
            sbias = smallp.tile([CS, 1], F32, tag="sbias")
            nc.scalar.copy(sbias[:], sb_ps[:])
            SBIAS[s] = sbias

        def ph_xsel(s):
            # x_sel = W_sel @ x + S b'  (fp16, straight from x)
            xsel = xselp.tile([CS, PIX], F16)
            for ntp in range(NT // 2):
                xps = [psMM.tile([CS, 512], F32, tag="mm", name=f"xps{j}") for j in range(2)]
                for kt in range(2):
                    for j in range(2):
                        nt = ntp * 2 + j
                        nc.tensor.matmul(
                            xps[j][:], lhsT=WSEL[s][kt][:],
                            rhs=X16[s][kt][:, nt * 512:(nt + 1) * 512],
                            start=(kt == 0), stop=(kt == 1))
                for j in range(2):
                    nt = ntp * 2 + j
                    nc.scalar.activation(xsel[:, nt * 512:(nt + 1) * 512],
                                         xps[j][:], AF.Identity,
                                         bias=SBIAS[s][:], scale=1.0)
            XSEL[s] = xsel
            # xsq_aug: rows 0-31 = xsel^2 (fp16), row 32 = 1.0 (carries the
            # -shift row of pQ through the Q matmul)
            xsq = xselp.tile([CS + 1, PIX], F16, tag="xsq", name="xsq")
            dma(out=xsq[CS:CS + 1, :], in_=ones_d[0:1, :])
            for h in range(2):
                nc.scalar.activation(xsq[0:CS, h * HPIX:(h + 1) * HPIX],
                                     xsel[:, h * HPIX:(h + 1) * HPIX],
                                     AF.Square)
            XSQ[s] = xsq
            if dbg:
                dma(out=dbg_xsel[s * CS:(s + 1) * CS, :], in_=xsel[:])

        def ph_z(s, m, half):
            # z = Square(sqrt(s/2)(xi+xj) @ pS) - [ (s/2)(xi^2+xj^2) - shift ]
            # PE: S and Q matmuls; ACT: Square (doubles as PSUM->SBUF move);
            # DVE: one subtract into the output staging tile.
            p = min(128, CSE - m * 128)
            xsel = XSEL[s]
            xsq = XSQ[s]
            zo = zp.tile([128, HPIX], F32)
            for ntp in range(2):
                sps = [psS.tile([128, 512], F32, tag="sps", name=f"sps{j}")
                       for j in range(2)]
                qps = [psQ.tile([128, 512], F32, tag="qps", name=f"qps{j}")
                       for j in range(2)]
                for j in range(2):
                    nt = half * 4 + ntp * 2 + j
                    nc.tensor.matmul(sps[j][:p, :],
                                     lhsT=pS16[:, m * 128: m * 128 + p],
                                     rhs=xsel[:, nt * 512:(nt + 1) * 512],
                                     start=True, stop=True)
                for j in range(2):
                    nt = half * 4 + ntp * 2 + j
                    nc.tensor.matmul(qps[j][:p, :],
                                     lhsT=pQ16[:, m * 128: m * 128 + p],
                                     rhs=xsq[:, nt * 512:(nt + 1) * 512],
                                     start=True, stop=True)
                for j in range(2):
                    nt2 = ntp * 2 + j
                    sq = sqp.tile([128, 512], F16, tag="sq", name="sq")
                    nc.scalar.activation(sq[:p, :], sps[j][:p, :], AF.Square)
                    # z = (Q * -1) + sq  — same operand pattern as the
                    # hardware-proven stt (PSUM f32 in0, fp16 SBUF in1)
                    nc.vector.scalar_tensor_tensor(
                        zo[:p, nt2 * 512:(nt2 + 1) * 512],
                        qps[j][:p, :], -1.0, sq[:p, :],
                        op0=ALU.mult, op1=ALU.add)
            if dbg:
                dma(out=dbg_t[s * 512 + m * 128: s * 512 + m * 128 + p,
                              half * HPIX:(half + 1) * HPIX],
                    in_=zo[:p, :])
            r0 = s * COUT + C1 + m * 128
            dma(out=out_d[r0:r0 + p, half * HPIX:(half + 1) * HPIX],
                in_=zo[:p, :])

        # program order == scheduling priority: casts and selection chains
        # early, z phases as soon as each sample's xsel is ready, y matmuls
        # fill PE gaps (their inputs are ready from the start).
        ph_cast(0)
        ph_cast(1)
        ph_sel(0)
        ph_xsel(0)
        ph_sel(1)
        ph_z(0, 0, 0)
        ph_z(0, 0, 1)
        ph_y(0)
        ph_xsel(1)
        ph_z(0, 1, 0)
        ph_z(0, 1, 1)
        ph_z(1, 0, 0)
        ph_z(1, 0, 1)
        ph_y(1)
        ph_z(1, 1, 0)
        ph_z(1, 1, 1)
        for m, half in [(2, 0), (2, 1), (3, 0), (3, 1)]:
            ph_z(0, m, half)
            ph_z(1, m, half)


_CACHE = {}


def _get_nc(dbg: bool = False):
    key = f"nc{int(dbg)}"
    if key not in _CACHE:
        nc = bacc.Bacc("TRN2", target_bir_lowering=False, debug=False,
                       num_devices=NCORES)
        _build(nc, dbg=dbg)
        nc.compile()
        _CACHE[key] = nc
    return _CACHE[key]


def _host_params(w_fc, b_fc, g_x, b_x, m_x, v_x, eca_w, g_s, b_s, m_s, v_s):
    sx = (g_x / np.sqrt(v_x + EPS)).astype(np.float32)            # [256]
    Wp = (sx[:, None] * w_fc).astype(np.float32)                  # [o, c]
    bp = (sx * b_fc + b_x - m_x * sx).astype(np.float32)          # [256]

    cmat = np.zeros((C1, C1), np.float32)                         # [o, q]
    for k in range(5):
        d = k - 2                                                 # o - q
        for q in range(C1):
            o = q + d
            if 0 <= o < C1:
                cmat[o, q] = eca_w[k]

    tril = (np.arange(C1)[None, :] < np.arange(C1)[:, None]).astype(np.float32)

    hi, hj = np.triu_indices(CS, k=1)
    ss = (g_s / np.sqrt(v_s + EPS)).astype(np.float32)
    sh = (b_s - m_s * ss).astype(np.float32)
    # squares-trick pair matrices:
    #   S' = pS.T @ xsel with pS[i,pq] = sqrt(s/2) * [i in (hi,hj)]
    #   Q  = pQ.T @ [xsel^2; 1] with pQ[i,pq] = (s/2)*[i in (hi,hj)],
    #        pQ[32,pq] = -shift  =>  z = S'^2 - Q
    ar = np.arange(CSE)
    pS = np.zeros((CS, CSE), np.float32)
    pS[hi, ar] = 1.0
    pS[hj, ar] += 1.0
    pS16 = (pS * np.sqrt(ss / 2.0)[None, :]).astype(np.float16)
    pQ = np.zeros((CS + 1, CSE), np.float32)
    pQ[hi, ar] = 1.0
    pQ[hj, ar] += 1.0
    pQ[:CS] *= (ss / 2.0)[None, :]
    pQ[CS] = -sh
    pQ16 = pQ.astype(np.float16)

    return {
        "wyT16": Wp.T.astype(np.float16).copy(),
        "wyT32s": (Wp.T / PIX).astype(np.float32).copy(),
        "wfold16": Wp.astype(np.float16).copy(),
        "bcol": bp.reshape(C1, 1).copy(),
        "bcol16": bp.astype(np.float16).reshape(C1, 1).copy(),
        "cmat": cmat,
        "tril": tril,
        "offd": (1.0 - np.eye(C1, dtype=np.float32)),
        "iota32": np.tile(np.arange(CS, dtype=np.float32), (128, 1)).copy(),
        "pS16": pS16,
        "pQ16": pQ16,
        "scol": ss.reshape(CSE, 1).copy(),
        "shcol": sh.reshape(CSE, 1).copy(),
    }


def _semantic_params(inputs):
    return _host_params(
        np.asarray(inputs["w_fc"], np.float32),
        np.asarray(inputs["b_fc"], np.float32),
        np.asarray(inputs["bn_x_gamma"], np.float32),
        np.asarray(inputs["bn_x_beta"], np.float32),
        np.asarray(inputs["bn_x_mean"], np.float32),
        np.asarray(inputs["bn_x_var"], np.float32),
        np.asarray(inputs["eca_w"], np.float32),
        np.asarray(inputs["bn_s_gamma"], np.float32),
        np.asarray(inputs["bn_s_beta"], np.float32),
        np.asarray(inputs["bn_s_mean"], np.float32),
        np.asarray(inputs["bn_s_var"], np.float32),
    )


def _pack_blobs(P):
    """Pack semantic params into the 3 const blobs matching _build's layout."""
    cb32 = np.zeros((128, CB32), np.float32)
    for k in range(2):
        cb32[:, _WY32 + k * 256: _WY32 + (k + 1) * 256] = \
            P["wyT32s"][k * 128:(k + 1) * 128]
        cb32[:, _CMAT + k * 256: _CMAT + (k + 1) * 256] = \
            P["cmat"][k * 128:(k + 1) * 128]
        cb32[:, _TRIL + k * 256: _TRIL + (k + 1) * 256] = \
            P["tril"][k * 128:(k + 1) * 128]
        cb32[:, _OFFD + k * 256: _OFFD + (k + 1) * 256] = \
            P["offd"][k * 128:(k + 1) * 128]
        cb32[:, _BCOL + k] = P["bcol"][k * 128:(k + 1) * 128, 0]
    for m in range(MT4):
        p = min(128, CSE - m * 128)
        cb32[:p, _SCOL + m] = P["scol"][m * 128: m * 128 + p, 0]
        cb32[:p, _SHCOL + m] = P["shcol"][m * 128: m * 128 + p, 0]
    cb32[:, _IOTA:_IOTA + CS] = P["iota32"]

    cb16 = np.zeros((128, CB16), np.float16)
    for k in range(2):
        cb16[:, _WY16 + k * 256: _WY16 + (k + 1) * 256] = \
            P["wyT16"][k * 128:(k + 1) * 128]
        cb16[:, _WFOLD + k * 256: _WFOLD + (k + 1) * 256] = \
            P["wfold16"][k * 128:(k + 1) * 128]
        cb16[:, _BC16 + k] = P["bcol16"][k * 128:(k + 1) * 128, 0]

    cbp = np.zeros((CS + 1, 2 * CSE), np.float16)
    cbp[0:CS, 0:CSE] = P["pS16"]
    cbp[0:CS + 1, CSE:2 * CSE] = P["pQ16"]
    return {"cb32": cb32, "cb16": np.ascontiguousarray(cb16),
            "cbp": np.ascontiguousarray(cbp),
            "ones_pix": np.ones((1, PIX), np.float16)}


def _in_maps(inputs):
    x = np.ascontiguousarray(np.asarray(inputs["x"], np.float32))
    blobs = _pack_blobs(_semantic_params(inputs))
    maps = []
    for c in range(NCORES):
        shard = x[c * SPC:(c + 1) * SPC].reshape(SPC * C1, PIX)
        maps.append({"x": np.ascontiguousarray(shard), **blobs})
    return maps


def _ensure_ntff_hook():
    """The agent image lacks antenv.axon_hooks; synthesize it so
    run_bass_kernel_spmd(trace=True) can reach the NTFF profiler in
    libaxon_pjrt.so. Safe no-op if anything is missing."""
    try:
        import antenv.axon_hooks  # noqa: F401
        return
    except ImportError:
        pass
    try:
        import types
        import antenv
        from trn_agent_boot.trn_boot import _ntff_profile_via_ctypes
        hook = _ntff_profile_via_ctypes("/opt/axon/libaxon_pjrt.so")
        mod = types.ModuleType("antenv.axon_hooks")
        mod._hook = hook
        mod.get_axon_ntff_profile_hook = lambda: mod._hook
        mod.set_axon_ntff_profile_hook = lambda h: setattr(mod, "_hook", h)
        sys.modules["antenv.axon_hooks"] = mod
        antenv.axon_hooks = mod
    except Exception as e:  # pragma: no cover
        print(f"ntff hook shim failed: {e}", file=sys.stderr)


def run(inputs, trace=False, dbg=False):
    if trace:
        _ensure_ntff_hook()
    nc = _get_nc(dbg=dbg)
    maps = _in_maps(inputs)
    res = run_bass_kernel_spmd(nc, maps, core_ids=list(range(NCORES)),
                               trace=trace)
    outs = [np.asarray(res.results[c]["out"], np.float32)
            .reshape(SPC, COUT, H, W) for c in range(NCORES)]
    return np.concatenate(outs, axis=0), res


def kernel(**inputs) -> np.ndarray:
    out, _ = run(inputs, trace=False)
    return out
